# revision 34
# baseline (speedup 1.0000x reference)
"""Tensor-parallel causal self-attention (RoPE) for 8 TRN2 NeuronCores.

Sharding: 16 heads -> 2 heads per core (TP). Each core computes
qkv projection for its heads (fp32r matmuls), RoPE, causal attention
(exp/softmax without max-subtraction -- scores are ~N(0,1)), and its
partial out-projection. The host sums the 8 partial outputs (the
all-reduce equivalent of TP out-projection).

Per-core layouts (host pre-transposes everything so no on-device
transposes are needed for the projections):
  xT    [C, B*T]  f32r   x^T, replicated on all cores
  wqkvT [C, 768]  f32r   columns = [q0,q1,k0,k1,v0,v1] head blocks,
                         q pre-scaled by 1/sqrt(D)
  woT   [256, C]  bf16   W_out columns for this core's heads, transposed
  cos2/sin2 [128, B*T] f32  RoPE tables duplicated in both halves
  out   [B*T, C]  f32    partial y (host sums over cores)
"""

import math
import sys

sys.path.insert(0, "/opt/trn_rl_repo")

import numpy as np

import concourse.bass as bass
import concourse.mybir as mybir
import concourse.tile as tile
from concourse import bacc
from concourse.bass import ds
from concourse.bass_utils import run_bass_kernel_spmd
from concourse.masks import make_causal_mask, make_identity

F32 = mybir.dt.float32
F32R = mybir.dt.float32r
BF16 = mybir.dt.bfloat16
EXP = mybir.ActivationFunctionType.Exp

B, T, C = 2, 2048, 2048
NH, D = 16, 128
NCORES, HPC = 8, 2          # heads per core
NTOK = B * T                # 4096
KB = C // 128               # 16 contraction blocks
NTC = NTOK // 512           # 8 token chunks of 512
NTB = NTOK // 128           # 32 token blocks of 128
QB = T // 128               # 16 query blocks per (b,h)


def build():
    nc = bacc.Bacc("TRN2", target_bir_lowering=False, debug=False,
                   num_devices=NCORES)
    xT = nc.dram_tensor("xT", [C, NTOK], F32R, kind="ExternalInput")
    wqkvT = nc.dram_tensor("wqkvT", [C, 3 * HPC * D], F32R, kind="ExternalInput")
    woT = nc.dram_tensor("woT", [HPC * D, C], BF16, kind="ExternalInput")
    cos2 = nc.dram_tensor("cos2", [128, NTOK], F32, kind="ExternalInput")
    sin2 = nc.dram_tensor("sin2", [128, NTOK], F32, kind="ExternalInput")
    out = nc.dram_tensor("out", [NTOK, C], F32, kind="ExternalOutput")

    with tile.TileContext(nc) as tc:
        with tc.tile_pool(name="const", bufs=1) as constp, \
             tc.tile_pool(name="qk", bufs=1) as qkp, \
             tc.tile_pool(name="v", bufs=1) as vp:
            ident_bf = constp.tile([128, 128], BF16, tag="idbf")
            make_identity(nc, ident_bf[:])
            ones_bf = constp.tile([128, 1], BF16, tag="onesbf")
            nc.gpsimd.memset(ones_bf[:], 1.0)
            # tri_mask[i, j] = 0 where j >= i else -30000: the transposed
            # causal mask for the diagonal 128x128 block of an S^T tile
            tri_mask = constp.tile([128, 128], BF16, tag="trimask")
            nc.gpsimd.memset(tri_mask[:], 0.0)
            nc.gpsimd.affine_select(
                out=tri_mask[:], in_=tri_mask[:],
                compare_op=mybir.AluOpType.is_ge,
                fill=-30000.0, base=0, pattern=[[1, 128]],
                channel_multiplier=-1)

            # q0,q1,k0,k1 in [d, tok] layout (f32r); v in [tok, d] block
            # layout (bf16)
            qk = [qkp.tile([128, NTOK], F32R, tag=f"qk{i}", name=f"qk{i}")
                  for i in range(4)]
            v_sb = [vp.tile([128, NTB * 128], BF16, tag=f"v{h}", name=f"v{h}")
                    for h in range(HPC)]

            # ---------------- phase 1: qkv projection + rope + v transpose
            with tc.tile_pool(name="w", bufs=1) as wp, \
                 tc.tile_pool(name="tab", bufs=2) as tabp, \
                 tc.tile_pool(name="x", bufs=2) as xp, \
                 tc.tile_pool(name="ps1", bufs=4, space="PSUM") as ps1, \
                 tc.tile_pool(name="tmp1", bufs=2, space="PSUM") as tmp1, \
                 tc.tile_pool(name="vtmp", bufs=2) as vtmpp, \
                 tc.tile_pool(name="vtps", bufs=2, space="PSUM") as vtps:
                # xT viewed as [p, kb, tok] so one DMA grabs 8 contraction
                # blocks of a token chunk at once. First chunk's x is issued
                # before the weights so the first matmul starts ASAP.
                xT_v = xT[:].rearrange("(g p) n -> p g n", p=128)
                x_tiles = {}
                for tci in (0,):
                    xa = xp.tile([128, 8, 512], F32R, tag="xa", name=f"xa{tci}")
                    xb = xp.tile([128, 8, 512], F32R, tag="xb", name=f"xb{tci}")
                    nc.sync.dma_start(xa[:], xT_v[:, 0:8, ds(tci * 512, 512)])
                    nc.sync.dma_start(xb[:], xT_v[:, 8:16, ds(tci * 512, 512)])
                    x_tiles[tci] = (xa, xb)

                w_sb = [wp.tile([128, 3 * HPC * D], F32R, tag=f"w{kb}", name=f"w{kb}")
                        for kb in range(KB)]
                for kb in range(KB):
                    nc.gpsimd.dma_start(w_sb[kb][:], wqkvT[ds(kb * 128, 128), :])

                def rope_or_v(mb, psum, tci, cos_sb, sin_sb):
                    s = ds(tci * 512, 512)
                    if mb < 4:
                        # rope: dst_lo = t1*cos - t2*sin,
                        #       dst_hi = t1*sin + t2*cos
                        dst = qk[mb]
                        tmp = tmp1.tile([128, 512], F32, tag="ropetmp",
                                        name=f"rt{tci}_{mb}")
                        nc.vector.tensor_mul(
                            tmp[0:64, :], psum[64:128, :], sin_sb[0:64, :])
                        nc.vector.tensor_mul(
                            tmp[64:128, :], psum[0:64, :], sin_sb[64:128, :])
                        nc.vector.tensor_mul(dst[:, s], psum[:], cos_sb[:])
                        nc.vector.tensor_sub(
                            dst[0:64, s], dst[0:64, s], tmp[0:64, :])
                        nc.vector.tensor_add(
                            dst[64:128, s], dst[64:128, s], tmp[64:128, :])
                    else:
                        h = mb - 4
                        vt = vtmpp.tile([128, 512], BF16, tag="vtmp",
                                        name=f"vt{tci}_{mb}")
                        nc.scalar.copy(vt[:], psum[:])
                        for j in range(4):
                            tp = vtps.tile([128, 128], BF16, tag="vtp",
                                           name=f"vtp{tci}_{mb}_{j}")
                            nc.tensor.transpose(
                                tp[:], vt[:, ds(j * 128, 128)], ident_bf[:])
                            tb = tci * 4 + j
                            nc.scalar.copy(
                                v_sb[h][:, ds(tb * 128, 128)], tp[:])

                for tci in range(NTC):
                    s = ds(tci * 512, 512)
                    cos_sb = tabp.tile([128, 512], F32, tag="cos",
                                       name=f"cos{tci}")
                    sin_sb = tabp.tile([128, 512], F32, tag="sin",
                                       name=f"sin{tci}")
                    nc.gpsimd.dma_start(cos_sb[:], cos2[:, s])
                    nc.gpsimd.dma_start(sin_sb[:], sin2[:, s])
                    if tci in x_tiles:
                        xa, xb = x_tiles[tci]
                    else:
                        xa = xp.tile([128, 8, 512], F32R, tag="xa",
                                     name=f"xa{tci}")
                        xb = xp.tile([128, 8, 512], F32R, tag="xb",
                                     name=f"xb{tci}")
                        nc.sync.dma_start(xa[:], xT_v[:, 0:8, s])
                        nc.sync.dma_start(xb[:], xT_v[:, 8:16, s])
                    for mb in range(6):
                        psum = ps1.tile([128, 512], F32, tag="qkvps",
                                        name=f"ps{tci}_{mb}")
                        for kb in range(KB):
                            xsrc = xa if kb < 8 else xb
                            nc.tensor.matmul(
                                psum[:], w_sb[kb][:, ds(mb * 128, 128)],
                                xsrc[:, kb % 8, :],
                                start=(kb == 0), stop=(kb == KB - 1))
                        rope_or_v(mb, psum, tci, cos_sb, sin_sb)

            # ---------------- phase 2: attention per (b, h), transposed
            # layout: S^T[tk, tq] = matmul(lhsT=kT block, rhs=qT chunk), so
            # softmax normalization happens on the output columns and P^T
            # feeds att@v directly (no P transposes). Rowsums come from a
            # ones-vector matmul over P^T; normalization is fused into the
            # O^T eviction.
            otp = tc.alloc_tile_pool(name="ot", bufs=1)
            ot_sb = [otp.tile([128, NTOK], BF16, tag=f"ot{h}", name=f"ot{h}")
                     for h in range(HPC)]
            with tc.tile_pool(name="sps", bufs=3, space="PSUM") as sps, \
                 tc.tile_pool(name="pt", bufs=4) as ptp_pool, \
                 tc.tile_pool(name="rsps", bufs=1, space="PSUM") as rsps, \
                 tc.tile_pool(name="fold", bufs=2) as foldp, \
                 tc.tile_pool(name="rrow", bufs=2) as rrowp, \
                 tc.tile_pool(name="rbc", bufs=2) as rbcp, \
                 tc.tile_pool(name="ops", bufs=4, space="PSUM") as opsp, \
                 tc.tile_pool(name="wo", bufs=1) as wop, \
                 tc.tile_pool(name="y", bufs=2) as ypool:
                wo_sb = [wop.tile([128, C], BF16, tag=f"wo{h}", name=f"wo{h}")
                         for h in range(HPC)]
                for h in range(HPC):
                    nc.gpsimd.dma_start(wo_sb[h][:], woT[ds(h * 128, 128), :])
                for b in range(B):
                    base = b * T
                    for h in range(HPC):
                        qT, kT = qk[h], qk[2 + h]
                        for g in range(4):          # tq chunk of 512
                            ntk = 4 * g + 4        # tk blocks covering chunk
                            o_ps = opsp.tile([128, 512], F32, tag="o")
                            rs_ps = rsps.tile([1, 512], F32, tag="rs")
                            for tkb in range(ntk):
                                r = tkb - 4 * g
                                lo = max(r, 0) * 128   # first causal column
                                n = 512 - lo
                                st = sps.tile([128, 512], F32, tag="s")
                                nc.tensor.matmul(
                                    st[:, ds(lo, n)],
                                    kT[:, ds(base + tkb * 128, 128)],
                                    qT[:, ds(base + g * 512 + lo, n)],
                                    start=True, stop=True)
                                if r >= 0:
                                    nc.vector.tensor_add(
                                        st[:, ds(lo, 128)], st[:, ds(lo, 128)],
                                        tri_mask[:])
                                pt = ptp_pool.tile([128, 512], BF16, tag="pt")
                                if lo:
                                    nc.gpsimd.memset(pt[:, ds(0, lo)], 0.0)
                                nc.scalar.activation(
                                    pt[:, ds(lo, n)], st[:, ds(lo, n)], EXP)
                                nc.tensor.matmul(
                                    rs_ps[:], ones_bf[:], pt[:],
                                    start=(tkb == 0), stop=(tkb == ntk - 1))
                                nc.tensor.matmul(
                                    o_ps[:],
                                    v_sb[h][:, ds((b * QB + tkb) * 128, 128)],
                                    pt[:], start=(tkb == 0), stop=(tkb == ntk - 1))
                            # normalization: broadcast the rowsum row,
                            # reciprocal over all partitions, multiply into
                            # the O^T eviction
                            rrow = rrowp.tile([1, 512], F32, tag="rrow")
                            nc.scalar.copy(rrow[:], rs_ps[:])
                            rrec = rrowp.tile([1, 512], F32, tag="rrec")
                            nc.vector.reciprocal_approx_fast(rrec[:], rrow[:])
                            rbc = rbcp.tile([128, 512], F32, tag="rbc")
                            nc.gpsimd.partition_broadcast(rbc[:], rrec[:])
                            nc.vector.tensor_mul(
                                ot_sb[h][:, ds(base + g * 512, 512)],
                                o_ps[:], rbc[:])

                    # ---------- out projection for this batch's tokens
                    # (shares the "o" psum tag so it overlaps the
                    # attention tail)
                    for tkl in range(QB):
                        tkb = b * QB + tkl
                        ysb = ypool.tile([128, C], F32, tag="y")
                        for pair in range(2):
                            yp2 = [opsp.tile([128, 512], F32, tag="o",
                                             name=f"yp{tkb}_{pair}_{oc}")
                                   for oc in range(2)]
                            for h in range(HPC):
                                for oc in range(2):
                                    nc.tensor.matmul(
                                        yp2[oc][:],
                                        ot_sb[h][:, ds(tkb * 128, 128)],
                                        wo_sb[h][:, ds((pair * 2 + oc) * 512, 512)],
                                        start=(h == 0), stop=(h == HPC - 1))
                            for oc in range(2):
                                col = ds((pair * 2 + oc) * 512, 512)
                                if oc % 2 == 0:
                                    nc.scalar.copy(ysb[:, col], yp2[oc][:])
                                else:
                                    nc.vector.tensor_copy(ysb[:, col], yp2[oc][:])
                        nc.sync.dma_start(out[ds(tkb * 128, 128), :], ysb[:])
            otp.release()

    nc.compile()
    return nc


_NC_CACHE = []


def _get_nc():
    if not _NC_CACHE:
        _NC_CACHE.append(build())
    return _NC_CACHE[0]


def make_in_maps(x, w_qkv, w_out):
    import ml_dtypes

    x2 = x.reshape(NTOK, C).astype(np.float32)
    xT = np.ascontiguousarray(x2.T)
    scale = np.float32(1.0 / math.sqrt(D))

    inv = 1.0 / (10000.0 ** (np.arange(0, D, 2, dtype=np.float32) / D))
    pos = np.arange(T, dtype=np.float32)
    ang = pos[:, None] * inv[None, :]            # [T, 64]
    cosT = np.cos(ang).T.astype(np.float32)      # [64, T]
    sinT = np.sin(ang).T.astype(np.float32)
    cos2 = np.tile(np.vstack([cosT, cosT]), (1, B))   # [128, NTOK]
    sin2 = np.tile(np.vstack([sinT, sinT]), (1, B))

    in_maps = []
    for c in range(NCORES):
        r = ds
        q = w_qkv[256 * c: 256 * (c + 1)] * scale
        k = w_qkv[C + 256 * c: C + 256 * (c + 1)]
        v = w_qkv[2 * C + 256 * c: 2 * C + 256 * (c + 1)]
        wl = np.concatenate([q, k, v], axis=0)       # [768, C]
        wqkvT = np.ascontiguousarray(wl.T.astype(np.float32))
        woT = np.ascontiguousarray(
            w_out[:, 256 * c: 256 * (c + 1)].T).astype(ml_dtypes.bfloat16)
        in_maps.append({
            "xT": xT, "wqkvT": wqkvT, "woT": woT,
            "cos2": cos2, "sin2": sin2,
        })
    return in_maps


def run(x, w_qkv, w_out, trace=False):
    nc = _get_nc()
    in_maps = make_in_maps(x, w_qkv, w_out)
    res = run_bass_kernel_spmd(nc, in_maps, core_ids=list(range(NCORES)),
                               trace=trace)
    y = res.results[0]["out"].astype(np.float32).copy()
    for i in range(1, NCORES):
        y += res.results[i]["out"]
    return y.reshape(B, T, C), res


def kernel(x, w_qkv, w_out):
    y, _ = run(x, w_qkv, w_out, trace=False)
    return y
